# revision 1
# baseline (speedup 1.0000x reference)
"""ConvAConnect TRN2 kernel: per-sample noisy-weight 3x3 conv, data-parallel over 8 cores.

Z[b] = conv2d_valid(X[b], W * Werr[loc_id[b]]) + bias * Berr[loc_id[b]]

Shapes: X[32,64,64,64] f32, W[3,3,64,128], bias[128], Werr[1000,3,3,64,128],
Berr[1000,128], loc_id[32] i32 -> Z[32,62,62,128] f32.

Strategy: shard batch (4 samples/core). Host prep = layout only (X transpose
to cin-major, gather of the 32 needed Werr/Berr pool rows, weight reshapes).
All FLOPs (memW = W*Werr, conv, bias) run on device.

Device kernel per sample:
  - Two stacked SBUF tiles of X^T (cin x H*W grid): XTs1 = [X^T; X^T shifted
    1 pixel], XTs64 = [X^T; X^T shifted 64 pixels]. The 9 conv taps collapse
    to 5 matmuls per 512-pixel output chunk: 4 with K=128 (tap pairs) and one
    K=64 single, accumulated in PSUM. fp32r dtype: full fp32 bits in memory,
    relaxed PE multiply (~2e-4 rel err), 2 cyc/row at N=512.
  - Output grid is 62 rows x 64 cols (2 junk columns keep width-64 alignment
    so every tap is a constant offset); junk columns are dropped at DMA time.
  - ScalarE copies PSUM->SBUF fused with the per-sample bias add; TensorE
    transposes [cout, spatial] -> [spatial, cout] in 128x128 blocks; the
    per-sample result is shipped with two 3D-AP DMAs (even/odd output rows),
    one on each HWDGE ring (sync + scalar).
"""

import sys
import numpy as np

for _p in ("/opt/trn_rl_repo", "/root/.axon_site"):
    if _p not in sys.path:
        sys.path.insert(0, _p)

N_CORES = 8
B = 32
PER_CORE = B // N_CORES
H = Wd = 64
CIN = 64
COUT = 128
HO = WO = 62
GRID = HO * 64          # 62 rows x 64 cols (2 junk cols/row)
XTL = 4104              # X^T free length: 4096 valid + pad (max read 4098)
NCHUNK = 512            # output-grid pixels per PSUM chunk (8 grid rows)
NCHUNKS = 8             # 7 full chunks + 1 of 384

_compiled = {}


def _build():
    import concourse.bass as bass
    import concourse.mybir as mybir
    import concourse.tile as tile
    from concourse import bacc
    from concourse.masks import make_identity

    f32 = mybir.dt.float32
    f32r = mybir.dt.float32r

    nc = bacc.Bacc("TRN2", target_bir_lowering=False, debug=False)

    xt_in = nc.dram_tensor("xt", [PER_CORE, CIN, XTL], f32r, kind="ExternalInput")
    wp_in = nc.dram_tensor("wp", [128, 3 * COUT], f32r, kind="ExternalInput")
    wq_in = nc.dram_tensor("wq", [128, COUT], f32r, kind="ExternalInput")
    ws_in = nc.dram_tensor("ws", [64, COUT], f32r, kind="ExternalInput")
    gp_in = nc.dram_tensor("gp", [PER_CORE, 128, 3 * COUT], f32r, kind="ExternalInput")
    gq_in = nc.dram_tensor("gq", [PER_CORE, 128, COUT], f32r, kind="ExternalInput")
    gs_in = nc.dram_tensor("gs", [PER_CORE, 64, COUT], f32r, kind="ExternalInput")
    bias_in = nc.dram_tensor("bias", [COUT, 1], f32, kind="ExternalInput")
    berr_in = nc.dram_tensor("berr", [COUT, PER_CORE], f32, kind="ExternalInput")
    z_out = nc.dram_tensor("z", [PER_CORE, 128, 31 * 128], f32, kind="ExternalOutput")

    with tile.TileContext(nc) as tc:
        with (
            tc.tile_pool(name="const", bufs=1) as const,
            tc.tile_pool(name="xpool", bufs=2) as xpool,
            tc.tile_pool(name="wpool", bufs=2) as wpool,
            tc.tile_pool(name="spool", bufs=4) as spool,
            tc.tile_pool(name="zpool", bufs=2) as zpool,
            tc.tile_pool(name="psmm", bufs=3, space="PSUM") as psmm,
            tc.tile_pool(name="pst", bufs=4, space="PSUM") as pst,
        ):
            identity = const.tile([128, 128], f32, tag="identity")
            make_identity(nc, identity[:])

            wp_t = const.tile([128, 3 * COUT], f32r, tag="wp")
            wq_t = const.tile([128, COUT], f32r, tag="wq")
            ws_t = const.tile([64, COUT], f32r, tag="ws")
            bias_t = const.tile([COUT, 1], f32, tag="bias")
            berr_t = const.tile([COUT, PER_CORE], f32, tag="berr")
            mb_all = const.tile([COUT, PER_CORE], f32, tag="mb")
            nc.sync.dma_start(wp_t[:], wp_in[:])
            nc.sync.dma_start(wq_t[:], wq_in[:])
            nc.sync.dma_start(ws_t[:], ws_in[:])
            nc.sync.dma_start(bias_t[:], bias_in[:])
            nc.sync.dma_start(berr_t[:], berr_in[:])
            nc.vector.tensor_scalar_mul(mb_all[:], berr_t[:], bias_t[:])

            def load_sample(b):
                """DMA the X stacks + noisy-weight factors and form memW."""
                xts1 = xpool.tile([128, XTL], f32r, tag="xts1")
                nc.sync.dma_start(xts1[0:64, :], xt_in[b])
                nc.scalar.dma_start(xts1[64:128, 0 : XTL - 1], xt_in[b][:, 1:XTL])
                xts64 = xpool.tile([128, XTL], f32r, tag="xts64")
                nc.scalar.dma_start(xts64[0:64, :], xt_in[b])
                nc.sync.dma_start(xts64[64:128, 0 : XTL - 64], xt_in[b][:, 64:XTL])

                gpt = wpool.tile([128, 3 * COUT], f32r, tag="gpt")
                gqt = wpool.tile([128, COUT], f32r, tag="gqt")
                gst = wpool.tile([64, COUT], f32r, tag="gst")
                nc.sync.dma_start(gpt[:], gp_in[b])
                nc.scalar.dma_start(gqt[:], gq_in[b])
                nc.scalar.dma_start(gst[:], gs_in[b])
                mwp = wpool.tile([128, 3 * COUT], f32r, tag="mwp")
                mwq = wpool.tile([128, COUT], f32r, tag="mwq")
                mws = wpool.tile([64, COUT], f32r, tag="mws")
                nc.vector.tensor_mul(mwp[:], wp_t[:], gpt[:])
                nc.vector.tensor_mul(mwq[:], wq_t[:], gqt[:])
                nc.vector.tensor_mul(mws[:], ws_t[:], gst[:])
                return xts1, xts64, mwp, mwq, mws

            cur = load_sample(0)
            for b in range(PER_CORE):
                xts1, xts64, mwp, mwq, mws = cur
                # prefetch next sample's inputs ahead of this sample's
                # output scatter so they don't queue behind it on the rings
                if b + 1 < PER_CORE:
                    cur = load_sample(b + 1)

                zbuf = zpool.tile([128, 31 * 128], f32, tag="zbuf")

                for c in range(NCHUNKS):
                    base = c * NCHUNK
                    ncols = min(NCHUNK, GRID - base)
                    pc = psmm.tile([128, NCHUNK], f32, tag="pc")
                    # taps (fh,0)+(fh,1): K=128 pairs from the shift-1 stack
                    for fh in range(3):
                        nc.tensor.matmul(
                            pc[:, :ncols],
                            mwp[:, fh * COUT : (fh + 1) * COUT],
                            xts1[:, base + fh * 64 : base + fh * 64 + ncols],
                            start=(fh == 0),
                            stop=False,
                        )
                    # taps (0,2)+(1,2): K=128 pair from the shift-64 stack
                    nc.tensor.matmul(
                        pc[:, :ncols],
                        mwq[:],
                        xts64[:, base + 2 : base + 2 + ncols],
                        start=False,
                        stop=False,
                    )
                    # tap (2,2): K=64 single (top half of xts1 is unshifted X^T)
                    nc.tensor.matmul(
                        pc[:, :ncols],
                        mws[:],
                        xts1[0:64, base + 130 : base + 130 + ncols],
                        start=False,
                        stop=True,
                    )
                    out_s = spool.tile([128, NCHUNK], f32, tag="outs")
                    nc.scalar.activation(
                        out_s[:, :ncols],
                        pc[:, :ncols],
                        mybir.ActivationFunctionType.Identity,
                        bias=mb_all[:, b : b + 1],
                    )
                    for k in range(ncols // 128):
                        pt = pst.tile([128, 128], f32, tag="pt")
                        nc.tensor.transpose(
                            pt[:], out_s[:, k * 128 : (k + 1) * 128], identity[:]
                        )
                        j = c * 4 + k  # grid row-pair index, 0..30
                        nc.vector.tensor_copy(
                            zbuf[:, j * 128 : (j + 1) * 128], pt[:]
                        )

                # ship the sample as one contiguous DMA in tiled layout
                # (partition p = 64*(ho%2)+wo, free = 128*(ho//2)+cout);
                # the host unshard does the final reshape/junk-drop
                eng = nc.sync if b % 2 == 0 else nc.scalar
                eng.dma_start(z_out[b], zbuf[:])

    nc.compile()
    return nc


def _get_nc():
    if "nc" not in _compiled:
        _compiled["nc"] = _build()
    return _compiled["nc"]


def _prep_inputs(X, W, bias, Werr, Berr, loc_id):
    """Host-side shard/layout prep. Returns per-core in_maps."""
    X = np.asarray(X, dtype=np.float32)
    W = np.asarray(W, dtype=np.float32)
    bias = np.asarray(bias, dtype=np.float32)
    Werr = np.asarray(Werr, dtype=np.float32)
    Berr = np.asarray(Berr, dtype=np.float32)
    loc_id = np.asarray(loc_id)

    # X^T: [B, CIN, H*W] padded to XTL
    xt = np.zeros((B, CIN, XTL), dtype=np.float32)
    xt[:, :, : H * Wd] = X.transpose(0, 3, 1, 2).reshape(B, CIN, H * Wd)

    # wp[fw*64+cin, fh*128+cout] = W[fh, fw, cin, cout] for fw in {0,1}
    wp = np.ascontiguousarray(W[:, :2].transpose(1, 2, 0, 3).reshape(128, 3 * COUT))
    # wq[fh*64+cin, cout] = W[fh, 2, cin, cout] for fh in {0,1}
    wq = np.ascontiguousarray(W[:2, 2].reshape(128, COUT))
    # ws[cin, cout] = W[2, 2, cin, cout]
    ws = np.ascontiguousarray(W[2, 2])

    g = Werr[loc_id]  # [B, 3, 3, 64, 128]
    gp = np.ascontiguousarray(
        g[:, :, :2].transpose(0, 2, 3, 1, 4).reshape(B, 128, 3 * COUT)
    )
    gq = np.ascontiguousarray(g[:, :2, 2].reshape(B, 128, COUT))
    gs = np.ascontiguousarray(g[:, 2, 2])

    be = Berr[loc_id]  # [B, 128]
    bias_col = np.ascontiguousarray(bias.reshape(COUT, 1))

    in_maps = []
    for i in range(N_CORES):
        s = slice(i * PER_CORE, (i + 1) * PER_CORE)
        in_maps.append(
            {
                "xt": np.ascontiguousarray(xt[s]),
                "wp": wp,
                "wq": wq,
                "ws": ws,
                "gp": np.ascontiguousarray(gp[s]),
                "gq": np.ascontiguousarray(gq[s]),
                "gs": np.ascontiguousarray(gs[s]),
                "bias": bias_col,
                "berr": np.ascontiguousarray(be[s].T),
            }
        )
    return in_maps


def _run(in_maps, trace=False, **kw):
    from concourse.bass_utils import run_bass_kernel_spmd

    nc = _get_nc()
    return run_bass_kernel_spmd(nc, in_maps, list(range(N_CORES)), trace=trace, **kw)


def _unshard(results):
    zb = np.concatenate([results[i]["z"] for i in range(N_CORES)], axis=0)
    # zb[b, 64*(ho%2)+wo, 128*(ho//2)+cout] -> Z[b, ho, wo, cout]
    v = zb.reshape(B, 2, 64, 31, COUT).transpose(0, 3, 1, 2, 4).reshape(B, HO, 64, COUT)
    return np.ascontiguousarray(v[:, :, :WO, :])


def kernel(X, W, bias, Werr, Berr, loc_id):
    in_maps = _prep_inputs(X, W, bias, Werr, Berr, loc_id)
    res = _run(in_maps)
    return _unshard(res.results)



# revision 6
# speedup vs baseline: 1.7112x; 1.7112x over previous
"""ConvAConnect TRN2 kernel: per-sample noisy-weight 3x3 conv, data-parallel over 8 cores.

Z[b] = conv2d_valid(X[b], W * Werr[loc_id[b]]) + bias * Berr[loc_id[b]]

Shapes: X[32,64,64,64] f32, W[3,3,64,128], bias[128], Werr[1000,3,3,64,128],
Berr[1000,128], loc_id[32] i32 -> Z[32,62,62,128] f32.

Strategy: shard batch (4 samples/core). Host prep = layout only (X transpose
to cin-major + fp16 cast + single shift-1 stack, gather of the 32 needed
Werr/Berr pool rows, weight packs). All FLOPs (memW = W*Werr, conv, bias)
run on device.

v2 design (vs 140us baseline, which was DMA-bound shipping X 4x at 64
partitions and burned PE time on output transposes):
  - X is shipped ONCE per sample as a 128-partition fp16 "shift-1 stack":
    rows 0-63 = X^T (cin x 4096 grid), rows 64-127 = X^T shifted 1 pixel.
    All 9 conv taps read this one buffer: tap pairs (0,1),(64,65),(128,129)
    are K=128 matmuls at col offsets 0/64/128; singles 2,66,130 are K=64
    matmuls (tap 66 reads the shifted bottom half at offset 65 with
    tile_position=(64,0)). 6 matmuls per 512-pixel PSUM chunk, fp16
    (1 cyc/row on PE, ~2^-12 element error).
  - No on-chip transpose: z stays [cout, spatial]; host does the final
    [spatial, cout] transpose + junk-column drop during unshard.
  - PSUM->SBUF copy fused with per-sample bias add rotates over ScalarE /
    PoolE / DVE; output shipped per sample as one fp16 DMA, rings alternate.
"""

import sys
import numpy as np
import ml_dtypes

for _p in ("/opt/trn_rl_repo", "/root/.axon_site"):
    if _p not in sys.path:
        sys.path.insert(0, _p)

N_CORES = 8
B = 32
PER_CORE = B // N_CORES
H = Wd = 64
CIN = 64
COUT = 128
HO = WO = 62
GRID = HO * 64          # 62 rows x 64 cols (2 junk cols/row)
XTL = 4104              # X^T free length: 4096 valid + pad (max read 4098)
NCHUNK = 512            # output-grid pixels per PSUM chunk (8 grid rows)
NCHUNKS = 8             # 7 full chunks + 1 of 384
WCOLS = 768             # weight pack: 3 K=128 pair blocks + 3 K=64 single blocks

_compiled = {}


def _build():
    import concourse.bass as bass
    import concourse.mybir as mybir
    import concourse.tile as tile
    from concourse import bacc

    f32 = mybir.dt.float32
    f16 = mybir.dt.float16

    nc = bacc.Bacc("TRN2", target_bir_lowering=False, debug=False)

    xt_in = nc.dram_tensor("xt", [PER_CORE, 128, XTL], f16, kind="ExternalInput")
    w_in = nc.dram_tensor("w", [128, WCOLS], f16, kind="ExternalInput")
    g_in = nc.dram_tensor("g", [PER_CORE, 128, WCOLS], f16, kind="ExternalInput")
    bias_in = nc.dram_tensor("bias", [COUT, 1], f32, kind="ExternalInput")
    berr_in = nc.dram_tensor("berr", [COUT, PER_CORE], f32, kind="ExternalInput")
    z_out = nc.dram_tensor("z", [PER_CORE, 128, GRID], f16, kind="ExternalOutput")

    with tile.TileContext(nc) as tc:
        with (
            tc.tile_pool(name="const", bufs=1) as const,
            tc.tile_pool(name="xpool", bufs=2) as xpool,
            tc.tile_pool(name="wpool", bufs=2) as wpool,
            tc.tile_pool(name="zpool", bufs=2) as zpool,
            tc.tile_pool(name="psmm", bufs=4, space="PSUM") as psmm,
        ):
            w_t = const.tile([128, WCOLS], f16, tag="w")
            bias_t = const.tile([COUT, 1], f32, tag="bias")
            berr_t = const.tile([COUT, PER_CORE], f32, tag="berr")
            mb_all = const.tile([COUT, PER_CORE], f32, tag="mb")
            nc.sync.dma_start(w_t[:], w_in[:])
            nc.sync.dma_start(bias_t[:], bias_in[:])
            nc.sync.dma_start(berr_t[:], berr_in[:])
            nc.vector.tensor_scalar_mul(mb_all[:], berr_t[:], bias_t[:])

            def load_sample(b):
                """DMA the X stack + noise pack, form memW = W*G on DVE."""
                xts = xpool.tile([128, XTL], f16, tag="xts")
                nc.sync.dma_start(xts[:], xt_in[b])
                gt = wpool.tile([128, WCOLS], f16, tag="gt")
                nc.scalar.dma_start(gt[:], g_in[b])
                mw = wpool.tile([128, WCOLS], f16, tag="mw")
                nc.vector.tensor_mul(mw[:], w_t[:], gt[:])
                return xts, mw

            cur = load_sample(0)
            for b in range(PER_CORE):
                xts, mw = cur
                # prefetch next sample's inputs ahead of this sample's
                # output DMA so they don't queue behind it on the rings
                if b + 1 < PER_CORE:
                    cur = load_sample(b + 1)

                zbuf = zpool.tile([128, GRID], f16, tag="zbuf")

                for c in range(NCHUNKS):
                    base = c * NCHUNK
                    n = min(NCHUNK, GRID - base)
                    pc = psmm.tile([128, NCHUNK], f32, tag="pc")
                    # tap pairs (0,1), (64,65), (128,129): K=128
                    for i in range(3):
                        nc.tensor.matmul(
                            pc[:, :n],
                            mw[:, i * 128 : (i + 1) * 128],
                            xts[:, base + i * 64 : base + i * 64 + n],
                            start=(i == 0),
                            stop=False,
                        )
                    # single taps 2, 66, 130: K=64 reads of the unshifted
                    # top half at the tap's grid offset
                    for j, off in enumerate((2, 66, 130)):
                        nc.tensor.matmul(
                            pc[:, :n],
                            mw[0:64, 384 + j * 128 : 512 + j * 128],
                            xts[0:64, base + off : base + off + n],
                            start=False,
                            stop=(j == 2),
                        )
                    # PSUM -> SBUF with fused per-sample bias add; alternate
                    # ScalarE / DVE (the only engines with a PSUM read port)
                    if c % 2 == 0:
                        nc.scalar.activation(
                            zbuf[:, base : base + n],
                            pc[:, :n],
                            mybir.ActivationFunctionType.Identity,
                            bias=mb_all[:, b : b + 1],
                        )
                    else:
                        nc.vector.tensor_scalar_add(
                            zbuf[:, base : base + n], pc[:, :n], mb_all[:, b : b + 1]
                        )

                eng = nc.sync if b % 2 == 0 else nc.scalar
                eng.dma_start(z_out[b], zbuf[:])

    nc.compile()
    return nc


def _get_nc():
    if "nc" not in _compiled:
        _compiled["nc"] = _build()
    return _compiled["nc"]


def _prep_inputs(X, W, bias, Werr, Berr, loc_id):
    """Host-side shard/layout prep. Returns per-core in_maps."""
    X = np.asarray(X, dtype=np.float32)
    W = np.asarray(W, dtype=np.float32)
    bias = np.asarray(bias, dtype=np.float32)
    Werr = np.asarray(Werr, dtype=np.float32)
    Berr = np.asarray(Berr, dtype=np.float32)
    loc_id = np.asarray(loc_id)

    # X^T shift-1 stack: [b, 0:64, j] = XT[b, cin, j]; [b, 64:128, j] = XT[., j+1]
    xt = X.transpose(0, 3, 1, 2).reshape(B, CIN, H * Wd).astype(np.float16)
    xts = np.zeros((B, 128, XTL), dtype=np.float16)
    xts[:, 0:64, : H * Wd] = xt
    xts[:, 64:128, : H * Wd - 1] = xt[:, :, 1:]

    def pack(w):
        # w: [..., 3, 3, 64, 128] -> [..., 128, 640] in the 6-block layout
        lead = w.shape[:-4]
        p = np.zeros(lead + (128, WCOLS), dtype=np.float16)
        for fh in range(3):  # K=128 pair blocks: taps (fh,0) + (fh,1)
            p[..., 0:64, fh * 128 : (fh + 1) * 128] = w[..., fh, 0, :, :]
            p[..., 64:128, fh * 128 : (fh + 1) * 128] = w[..., fh, 1, :, :]
        p[..., 0:64, 384:512] = w[..., 0, 2, :, :]   # tap 2
        p[..., 0:64, 512:640] = w[..., 1, 2, :, :]   # tap 66
        p[..., 0:64, 640:768] = w[..., 2, 2, :, :]   # tap 130
        return p

    wpack = pack(W)
    gpack = pack(Werr[loc_id])  # [B, 128, 640]

    be = Berr[loc_id]  # [B, 128]
    bias_col = np.ascontiguousarray(bias.reshape(COUT, 1))

    in_maps = []
    for i in range(N_CORES):
        s = slice(i * PER_CORE, (i + 1) * PER_CORE)
        in_maps.append(
            {
                "xt": np.ascontiguousarray(xts[s]),
                "w": wpack,
                "g": np.ascontiguousarray(gpack[s]),
                "bias": bias_col,
                "berr": np.ascontiguousarray(be[s].T),
            }
        )
    return in_maps


def _run(in_maps, trace=False, **kw):
    from concourse.bass_utils import run_bass_kernel_spmd

    nc = _get_nc()
    return run_bass_kernel_spmd(nc, in_maps, list(range(N_CORES)), trace=trace, **kw)


def _unshard(results):
    zb = np.concatenate([results[i]["z"] for i in range(N_CORES)], axis=0)
    # zb[b, cout, grid] fp16 -> Z[b, ho, wo, cout] f32, dropping 2 junk cols
    v = zb.astype(np.float32).reshape(B, COUT, HO, 64).transpose(0, 2, 3, 1)
    return np.ascontiguousarray(v[:, :, :WO, :])


def kernel(X, W, bias, Werr, Berr, loc_id):
    in_maps = _prep_inputs(X, W, bias, Werr, Berr, loc_id)
    res = _run(in_maps)
    return _unshard(res.results)


# revision 7
# speedup vs baseline: 1.9252x; 1.1250x over previous
"""ConvAConnect TRN2 kernel: per-sample noisy-weight 3x3 conv, data-parallel over 8 cores.

Z[b] = conv2d_valid(X[b], W * Werr[loc_id[b]]) + bias * Berr[loc_id[b]]

Shapes: X[32,64,64,64] f32, W[3,3,64,128], bias[128], Werr[1000,3,3,64,128],
Berr[1000,128], loc_id[32] i32 -> Z[32,62,62,128] f32.

Strategy: shard batch (4 samples/core). Host prep = layout only (X transpose
to cin-major + fp16 cast + two shifted stacks, gather of the 32 needed
Werr/Berr pool rows, weight packs). All FLOPs (memW = W*Werr, conv, bias)
run on device.

v4 design (baseline was 140us, DMA-bound on 64-partition X transfers and
PE-bound on output transposes; v3 at 82us was PE-bound at 6 matmuls/chunk):
  - X ships as TWO 128-partition fp16 stacks per sample, one DMA each on
    separate HWDGE rings: xts1 = [X^T; X^T shifted 1 px], xts64 = [X^T;
    X^T shifted 64 px]. The 9 conv taps then need only 5 matmuls per
    512-pixel PSUM chunk: tap pairs (0,1),(64,65),(128,129) from xts1 at
    col offsets 0/64/128, pair (2,66) from xts64 at offset 2, single 130
    as K=64 from the unshifted xts1 top half. fp16 operands: 1 cyc/row on
    PE, ~2^-12 element error, and back-to-back matmuls pipeline at full
    stream rate.
  - No on-chip transpose: z stays [cout, spatial] fp16; host does the final
    [spatial, cout] transpose + junk-column drop + f32 upcast in unshard.
  - PSUM->SBUF copy fused with the per-sample bias add alternates between
    ScalarE and DVE (the only PSUM-capable engines); z ships in two half
    DMAs per sample (issued as soon as their chunks finish) on alternating
    rings.
  - Startup is latency-ordered: sample-0 X stacks are split into two
    column-half DMAs per ring so the first chunk's matmuls start after
    ~a quarter of the X bytes have landed.
"""

import sys
import numpy as np

for _p in ("/opt/trn_rl_repo", "/root/.axon_site"):
    if _p not in sys.path:
        sys.path.insert(0, _p)

N_CORES = 8
B = 32
PER_CORE = B // N_CORES
H = Wd = 64
CIN = 64
COUT = 128
HO = WO = 62
GRID = HO * 64          # 62 rows x 64 cols (2 junk cols/row)
XTL = 4104              # X^T free length: 4096 valid + pad (max read 4098)
XHALF = 2052
NCHUNK = 512            # output-grid pixels per PSUM chunk (8 grid rows)
NCHUNKS = 8             # 7 full chunks + 1 of 384
WCOLS = 640             # 3 K=128 pair blocks + pair(2,66) block + K=64 blk 130

_compiled = {}


def _build():
    import concourse.bass as bass
    import concourse.mybir as mybir
    import concourse.tile as tile
    from concourse import bacc

    f32 = mybir.dt.float32
    f16 = mybir.dt.float16

    nc = bacc.Bacc("TRN2", target_bir_lowering=False, debug=False)

    x1_in = nc.dram_tensor("x1", [PER_CORE, 128, XTL], f16, kind="ExternalInput")
    x64_in = nc.dram_tensor("x64", [PER_CORE, 128, XTL], f16, kind="ExternalInput")
    w_in = nc.dram_tensor("w", [128, WCOLS], f16, kind="ExternalInput")
    g_in = nc.dram_tensor("g", [PER_CORE, 128, WCOLS], f16, kind="ExternalInput")
    bias_in = nc.dram_tensor("bias", [COUT, 1], f32, kind="ExternalInput")
    berr_in = nc.dram_tensor("berr", [COUT, PER_CORE], f32, kind="ExternalInput")
    z_out = nc.dram_tensor("z", [PER_CORE, 128, GRID], f16, kind="ExternalOutput")

    with tile.TileContext(nc) as tc:
        with (
            tc.tile_pool(name="const", bufs=1) as const,
            tc.tile_pool(name="xpool", bufs=2) as xpool,
            tc.tile_pool(name="wpool", bufs=2) as wpool,
            tc.tile_pool(name="zpool", bufs=2) as zpool,
            tc.tile_pool(name="psmm", bufs=4, space="PSUM") as psmm,
        ):
            w_t = const.tile([128, WCOLS], f16, tag="w")
            bias_t = const.tile([COUT, 1], f32, tag="bias")
            berr_t = const.tile([COUT, PER_CORE], f32, tag="berr")
            mb_all = const.tile([COUT, PER_CORE], f32, tag="mb")

            def load_sample(b, first=False):
                """DMA the X stacks + noise pack, form memW = W*G on DVE.

                Sample 0 is latency-critical: its X DMAs are split into
                column halves so chunk 0 can start early, and the tiny
                w/g/bias transfers are interleaved ahead of the bulk.
                """
                xts1 = xpool.tile([128, XTL], f16, tag="xts1")
                xts64 = xpool.tile([128, XTL], f16, tag="xts64")
                gt = wpool.tile([128, WCOLS], f16, tag="gt")
                mw = wpool.tile([128, WCOLS], f16, tag="mw")
                if first:
                    nc.scalar.dma_start(w_t[:], w_in[:])
                    nc.sync.dma_start(gt[:], g_in[b])
                    nc.sync.dma_start(xts1[:, 0:XHALF], x1_in[b][:, 0:XHALF])
                    nc.scalar.dma_start(xts64[:, 0:XHALF], x64_in[b][:, 0:XHALF])
                    nc.vector.tensor_mul(mw[:], w_t[:], gt[:])
                    nc.sync.dma_start(xts1[:, XHALF:XTL], x1_in[b][:, XHALF:XTL])
                    nc.scalar.dma_start(xts64[:, XHALF:XTL], x64_in[b][:, XHALF:XTL])
                    nc.sync.dma_start(bias_t[:], bias_in[:])
                    nc.sync.dma_start(berr_t[:], berr_in[:])
                    nc.vector.tensor_scalar_mul(mb_all[:], berr_t[:], bias_t[:])
                else:
                    nc.sync.dma_start(xts1[:], x1_in[b])
                    nc.scalar.dma_start(xts64[:], x64_in[b])
                    nc.scalar.dma_start(gt[:], g_in[b])
                    nc.vector.tensor_mul(mw[:], w_t[:], gt[:])
                return xts1, xts64, mw

            cur = load_sample(0, first=True)
            for b in range(PER_CORE):
                xts1, xts64, mw = cur
                if b + 1 < PER_CORE:
                    cur = load_sample(b + 1)

                zbuf = zpool.tile([128, GRID], f16, tag="zbuf")

                for c in range(NCHUNKS):
                    base = c * NCHUNK
                    n = min(NCHUNK, GRID - base)
                    pc = psmm.tile([128, NCHUNK], f32, tag="pc")
                    # tap pairs (0,1), (64,65), (128,129): K=128 from xts1
                    for i in range(3):
                        nc.tensor.matmul(
                            pc[:, :n],
                            mw[:, i * 128 : (i + 1) * 128],
                            xts1[:, base + i * 64 : base + i * 64 + n],
                            start=(i == 0),
                            stop=False,
                        )
                    # tap pair (2,66): K=128 from xts64 at offset 2
                    nc.tensor.matmul(
                        pc[:, :n],
                        mw[:, 384:512],
                        xts64[:, base + 2 : base + 2 + n],
                        start=False,
                        stop=False,
                    )
                    # tap 130: K=64 from the unshifted xts1 top half
                    nc.tensor.matmul(
                        pc[:, :n],
                        mw[0:64, 512:640],
                        xts1[0:64, base + 130 : base + 130 + n],
                        start=False,
                        stop=True,
                    )
                    # PSUM -> SBUF with fused per-sample bias add; alternate
                    # ScalarE / DVE (the only engines with a PSUM read port)
                    if c % 2 == 0:
                        nc.scalar.activation(
                            zbuf[:, base : base + n],
                            pc[:, :n],
                            mybir.ActivationFunctionType.Identity,
                            bias=mb_all[:, b : b + 1],
                        )
                    else:
                        nc.vector.tensor_scalar_add(
                            zbuf[:, base : base + n], pc[:, :n], mb_all[:, b : b + 1]
                        )
                    # ship finished output halves as soon as they complete
                    if c == 3:
                        nc.sync.dma_start(
                            z_out[b][:, 0 : 4 * NCHUNK], zbuf[:, 0 : 4 * NCHUNK]
                        )
                    elif c == NCHUNKS - 1:
                        nc.scalar.dma_start(
                            z_out[b][:, 4 * NCHUNK : GRID], zbuf[:, 4 * NCHUNK : GRID]
                        )

    nc.compile()
    return nc


def _get_nc():
    if "nc" not in _compiled:
        _compiled["nc"] = _build()
    return _compiled["nc"]


def _prep_inputs(X, W, bias, Werr, Berr, loc_id):
    """Host-side shard/layout prep. Returns per-core in_maps."""
    X = np.asarray(X, dtype=np.float32)
    W = np.asarray(W, dtype=np.float32)
    bias = np.asarray(bias, dtype=np.float32)
    Werr = np.asarray(Werr, dtype=np.float32)
    Berr = np.asarray(Berr, dtype=np.float32)
    loc_id = np.asarray(loc_id)

    # X^T stacks: x1 = [X^T; X^T shifted 1 col], x64 = [X^T; X^T shifted 64]
    xt = X.transpose(0, 3, 1, 2).reshape(B, CIN, H * Wd).astype(np.float16)
    x1 = np.zeros((B, 128, XTL), dtype=np.float16)
    x1[:, 0:64, : H * Wd] = xt
    x1[:, 64:128, : H * Wd - 1] = xt[:, :, 1:]
    x64 = np.zeros((B, 128, XTL), dtype=np.float16)
    x64[:, 0:64, : H * Wd] = xt
    x64[:, 64:128, : H * Wd - 64] = xt[:, :, 64:]

    def pack(w):
        # w: [..., 3, 3, 64, 128] -> [..., 128, 640]
        lead = w.shape[:-4]
        p = np.zeros(lead + (128, WCOLS), dtype=np.float16)
        for fh in range(3):  # K=128 pair blocks: taps (fh,0) + (fh,1)
            p[..., 0:64, fh * 128 : (fh + 1) * 128] = w[..., fh, 0, :, :]
            p[..., 64:128, fh * 128 : (fh + 1) * 128] = w[..., fh, 1, :, :]
        p[..., 0:64, 384:512] = w[..., 0, 2, :, :]   # tap 2 (xts64 top)
        p[..., 64:128, 384:512] = w[..., 1, 2, :, :]  # tap 66 (xts64 bottom)
        p[..., 0:64, 512:640] = w[..., 2, 2, :, :]   # tap 130 (K=64)
        return p

    wpack = pack(W)
    gpack = pack(Werr[loc_id])  # [B, 128, 640]

    be = Berr[loc_id]  # [B, 128]
    bias_col = np.ascontiguousarray(bias.reshape(COUT, 1))

    in_maps = []
    for i in range(N_CORES):
        s = slice(i * PER_CORE, (i + 1) * PER_CORE)
        in_maps.append(
            {
                "x1": np.ascontiguousarray(x1[s]),
                "x64": np.ascontiguousarray(x64[s]),
                "w": wpack,
                "g": np.ascontiguousarray(gpack[s]),
                "bias": bias_col,
                "berr": np.ascontiguousarray(be[s].T),
            }
        )
    return in_maps


def _run(in_maps, trace=False, **kw):
    from concourse.bass_utils import run_bass_kernel_spmd

    nc = _get_nc()
    return run_bass_kernel_spmd(nc, in_maps, list(range(N_CORES)), trace=trace, **kw)


def _unshard(results):
    zb = np.concatenate([results[i]["z"] for i in range(N_CORES)], axis=0)
    # zb[b, cout, grid] fp16 -> Z[b, ho, wo, cout] f32, dropping 2 junk cols
    v = zb.astype(np.float32).reshape(B, COUT, HO, 64).transpose(0, 2, 3, 1)
    return np.ascontiguousarray(v[:, :, :WO, :])


def kernel(X, W, bias, Werr, Berr, loc_id):
    in_maps = _prep_inputs(X, W, bias, Werr, Berr, loc_id)
    res = _run(in_maps)
    return _unshard(res.results)


# revision 8
# speedup vs baseline: 2.1752x; 1.1299x over previous
"""ConvAConnect TRN2 kernel: per-sample noisy-weight 3x3 conv, data-parallel over 8 cores.

Z[b] = conv2d_valid(X[b], W * Werr[loc_id[b]]) + bias * Berr[loc_id[b]]

Shapes: X[32,64,64,64] f32, W[3,3,64,128], bias[128], Werr[1000,3,3,64,128],
Berr[1000,128], loc_id[32] i32 -> Z[32,62,62,128] f32.

Strategy: shard batch (4 samples/core). Host prep = layout only (X transpose
to cin-major + fp16 cast + two shifted stacks, gather of the 32 needed
Werr/Berr pool rows, weight packs). All FLOPs (memW = W*Werr, conv, bias)
run on device.

v5 design notes (140us baseline -> 82us -> 72.5us -> this):
  - X ships as TWO 128-partition fp16 "shifted stacks" per sample (xts1 =
    [X^T; X^T<<1], xts64 = [X^T; X^T<<64]), each split into an A tile
    (grid cols 0-2303, feeds chunks 0-3) and a B tile (cols 2048-4103,
    feeds chunks 4-7) on separate HWDGE rings. Separate tiles make the
    first chunk's data dependency a quarter of the X bytes, so the PE
    starts ~6us earlier than with whole-stack tiles.
  - 5 matmuls per 512-pixel PSUM chunk, all fp16 (1 cyc/row, ~2^-12 err):
    tap pairs (0,1),(64,65),(128,129) from xts1 at col offsets 0/64/128,
    pair (2,66) from xts64 at offset 2, single 130 as K=64 from the
    unshifted xts1 top half. Back-to-back matmuls pipeline at stream rate.
  - No on-chip transpose: z stays [cout, spatial] fp16; host does the
    final [spatial, cout] transpose + junk-column drop + f32 upcast.
  - All PSUM->SBUF copies (fused bias add) run on DVE via tensor_scalar;
    ScalarE issues no compute at all, which drops the activation-table
    loads from the preamble and leaves the ACT ring free for DMA.
  - z ships as four quarter-DMAs per sample on alternating rings, each
    issued the moment its two chunks are copied, so the output tail after
    the last matmul is ~1us.
"""

import sys
import numpy as np

for _p in ("/opt/trn_rl_repo", "/root/.axon_site"):
    if _p not in sys.path:
        sys.path.insert(0, _p)

N_CORES = 8
B = 32
PER_CORE = B // N_CORES
H = Wd = 64
CIN = 64
COUT = 128
HO = WO = 62
GRID = HO * 64          # 62 rows x 64 cols (2 junk cols/row)
XTL = 4104              # X^T grid cols: 4096 valid + pad (max read 4098)
ACOLS = 2304            # A tile: grid cols [0, 2304), serves chunks 0-3
BOFF = 2048             # B tile: grid cols [2048, 4104), serves chunks 4-7
BCOLS = XTL - BOFF      # 2056
NCHUNK = 512            # output-grid pixels per PSUM chunk (8 grid rows)
NCHUNKS = 8             # 7 full chunks + 1 of 384
WCOLS = 640             # 3 K=128 pair blocks + pair(2,66) block + K=64 blk 130

_compiled = {}


def _build():
    import concourse.bass as bass
    import concourse.mybir as mybir
    import concourse.tile as tile
    from concourse import bacc

    f32 = mybir.dt.float32
    f16 = mybir.dt.float16

    nc = bacc.Bacc("TRN2", target_bir_lowering=False, debug=False)

    x1a_in = nc.dram_tensor("x1a", [PER_CORE, 128, ACOLS], f16, kind="ExternalInput")
    x1b_in = nc.dram_tensor("x1b", [PER_CORE, 128, BCOLS], f16, kind="ExternalInput")
    x64a_in = nc.dram_tensor("x64a", [PER_CORE, 128, ACOLS], f16, kind="ExternalInput")
    x64b_in = nc.dram_tensor("x64b", [PER_CORE, 128, BCOLS], f16, kind="ExternalInput")
    w_in = nc.dram_tensor("w", [128, WCOLS], f16, kind="ExternalInput")
    g_in = nc.dram_tensor("g", [PER_CORE, 128, WCOLS], f16, kind="ExternalInput")
    bias_in = nc.dram_tensor("bias", [COUT, 1], f32, kind="ExternalInput")
    berr_in = nc.dram_tensor("berr", [COUT, PER_CORE], f32, kind="ExternalInput")
    z_out = nc.dram_tensor("z", [PER_CORE, 128, GRID], f16, kind="ExternalOutput")

    with tile.TileContext(nc) as tc:
        with (
            tc.tile_pool(name="const", bufs=1) as const,
            tc.tile_pool(name="xpool", bufs=2) as xpool,
            tc.tile_pool(name="wpool", bufs=2) as wpool,
            tc.tile_pool(name="zpool", bufs=2) as zpool,
            tc.tile_pool(name="psmm", bufs=4, space="PSUM") as psmm,
        ):
            w_t = const.tile([128, WCOLS], f16, tag="w")
            bias_t = const.tile([COUT, 1], f32, tag="bias")
            berr_t = const.tile([COUT, PER_CORE], f32, tag="berr")
            mb_all = const.tile([COUT, PER_CORE], f32, tag="mb")

            def load_sample(b, first=False):
                """DMA the X stack tiles + noise pack, form memW = W*G on DVE.

                The A tiles (first four chunks' data) go first on both
                rings; sample 0 additionally front-loads the small w/g
                packs so memW is ready before x1a lands.
                """
                x1at = xpool.tile([128, ACOLS], f16, tag="x1a")
                x1bt = xpool.tile([128, BCOLS], f16, tag="x1b")
                x64at = xpool.tile([128, ACOLS], f16, tag="x64a")
                x64bt = xpool.tile([128, BCOLS], f16, tag="x64b")
                gt = wpool.tile([128, WCOLS], f16, tag="gt")
                mw = wpool.tile([128, WCOLS], f16, tag="mw")
                if first:
                    nc.scalar.dma_start(w_t[:], w_in[:])
                    nc.sync.dma_start(gt[:], g_in[b])
                    nc.sync.dma_start(x1at[:], x1a_in[b])
                    nc.scalar.dma_start(x64at[:], x64a_in[b])
                    nc.vector.tensor_mul(mw[:], w_t[:], gt[:])
                    nc.sync.dma_start(x1bt[:], x1b_in[b])
                    nc.scalar.dma_start(x64bt[:], x64b_in[b])
                    nc.sync.dma_start(bias_t[:], bias_in[:])
                    nc.sync.dma_start(berr_t[:], berr_in[:])
                    nc.vector.tensor_scalar_mul(mb_all[:], berr_t[:], bias_t[:])
                else:
                    nc.sync.dma_start(x1at[:], x1a_in[b])
                    nc.scalar.dma_start(x64at[:], x64a_in[b])
                    nc.sync.dma_start(x1bt[:], x1b_in[b])
                    nc.scalar.dma_start(x64bt[:], x64b_in[b])
                    nc.scalar.dma_start(gt[:], g_in[b])
                    nc.vector.tensor_mul(mw[:], w_t[:], gt[:])
                return x1at, x1bt, x64at, x64bt, mw

            cur = load_sample(0, first=True)
            for b in range(PER_CORE):
                x1at, x1bt, x64at, x64bt, mw = cur
                if b + 1 < PER_CORE:
                    cur = load_sample(b + 1)

                zbuf = zpool.tile([128, GRID], f16, tag="zbuf")

                for c in range(NCHUNKS):
                    base = c * NCHUNK
                    n = min(NCHUNK, GRID - base)
                    x1, x64 = (x1at, x64at) if c < 4 else (x1bt, x64bt)
                    off = base if c < 4 else base - BOFF
                    pc = psmm.tile([128, NCHUNK], f32, tag="pc")
                    # tap pairs (0,1), (64,65), (128,129): K=128 from xts1
                    for i in range(3):
                        nc.tensor.matmul(
                            pc[:, :n],
                            mw[:, i * 128 : (i + 1) * 128],
                            x1[:, off + i * 64 : off + i * 64 + n],
                            start=(i == 0),
                            stop=False,
                        )
                    # tap pair (2,66): K=128 from xts64 at offset 2
                    nc.tensor.matmul(
                        pc[:, :n],
                        mw[:, 384:512],
                        x64[:, off + 2 : off + 2 + n],
                        start=False,
                        stop=False,
                    )
                    # tap 130: K=64 from the unshifted xts1 top half
                    nc.tensor.matmul(
                        pc[:, :n],
                        mw[0:64, 512:640],
                        x1[0:64, off + 130 : off + 130 + n],
                        start=False,
                        stop=True,
                    )
                    # PSUM -> SBUF with fused per-sample bias add on DVE
                    nc.vector.tensor_scalar_add(
                        zbuf[:, base : base + n], pc[:, :n], mb_all[:, b : b + 1]
                    )
                    # ship each finished output quarter immediately
                    if c % 2 == 1:
                        q = c // 2
                        lo, hi = q * 2 * NCHUNK, min((q + 1) * 2 * NCHUNK, GRID)
                        eng = nc.sync if q % 2 == 0 else nc.scalar
                        eng.dma_start(z_out[b][:, lo:hi], zbuf[:, lo:hi])

    nc.compile()
    return nc


def _get_nc():
    if "nc" not in _compiled:
        _compiled["nc"] = _build()
    return _compiled["nc"]


def _prep_inputs(X, W, bias, Werr, Berr, loc_id):
    """Host-side shard/layout prep. Returns per-core in_maps."""
    X = np.asarray(X, dtype=np.float32)
    W = np.asarray(W, dtype=np.float32)
    bias = np.asarray(bias, dtype=np.float32)
    Werr = np.asarray(Werr, dtype=np.float32)
    Berr = np.asarray(Berr, dtype=np.float32)
    loc_id = np.asarray(loc_id)

    # X^T stacks: x1 = [X^T; X^T shifted 1 col], x64 = [X^T; X^T shifted 64]
    xt = X.transpose(0, 3, 1, 2).reshape(B, CIN, H * Wd).astype(np.float16)
    x1 = np.zeros((B, 128, XTL), dtype=np.float16)
    x1[:, 0:64, : H * Wd] = xt
    x1[:, 64:128, : H * Wd - 1] = xt[:, :, 1:]
    x64 = np.zeros((B, 128, XTL), dtype=np.float16)
    x64[:, 0:64, : H * Wd] = xt
    x64[:, 64:128, : H * Wd - 64] = xt[:, :, 64:]

    def pack(w):
        # w: [..., 3, 3, 64, 128] -> [..., 128, 640]
        lead = w.shape[:-4]
        p = np.zeros(lead + (128, WCOLS), dtype=np.float16)
        for fh in range(3):  # K=128 pair blocks: taps (fh,0) + (fh,1)
            p[..., 0:64, fh * 128 : (fh + 1) * 128] = w[..., fh, 0, :, :]
            p[..., 64:128, fh * 128 : (fh + 1) * 128] = w[..., fh, 1, :, :]
        p[..., 0:64, 384:512] = w[..., 0, 2, :, :]   # tap 2 (xts64 top)
        p[..., 64:128, 384:512] = w[..., 1, 2, :, :]  # tap 66 (xts64 bottom)
        p[..., 0:64, 512:640] = w[..., 2, 2, :, :]   # tap 130 (K=64)
        return p

    wpack = pack(W)
    gpack = pack(Werr[loc_id])  # [B, 128, 640]

    be = Berr[loc_id]  # [B, 128]
    bias_col = np.ascontiguousarray(bias.reshape(COUT, 1))

    in_maps = []
    for i in range(N_CORES):
        s = slice(i * PER_CORE, (i + 1) * PER_CORE)
        in_maps.append(
            {
                "x1a": np.ascontiguousarray(x1[s, :, :ACOLS]),
                "x1b": np.ascontiguousarray(x1[s, :, BOFF:]),
                "x64a": np.ascontiguousarray(x64[s, :, :ACOLS]),
                "x64b": np.ascontiguousarray(x64[s, :, BOFF:]),
                "w": wpack,
                "g": np.ascontiguousarray(gpack[s]),
                "bias": bias_col,
                "berr": np.ascontiguousarray(be[s].T),
            }
        )
    return in_maps


def _run(in_maps, trace=False, **kw):
    from concourse.bass_utils import run_bass_kernel_spmd

    nc = _get_nc()
    return run_bass_kernel_spmd(nc, in_maps, list(range(N_CORES)), trace=trace, **kw)


def _unshard(results):
    zb = np.concatenate([results[i]["z"] for i in range(N_CORES)], axis=0)
    # zb[b, cout, grid] fp16 -> Z[b, ho, wo, cout] f32, dropping 2 junk cols
    v = zb.astype(np.float32).reshape(B, COUT, HO, 64).transpose(0, 2, 3, 1)
    return np.ascontiguousarray(v[:, :, :WO, :])


def kernel(X, W, bias, Werr, Berr, loc_id):
    in_maps = _prep_inputs(X, W, bias, Werr, Berr, loc_id)
    res = _run(in_maps)
    return _unshard(res.results)


# revision 14
# speedup vs baseline: 2.1796x; 1.0020x over previous
"""ConvAConnect TRN2 kernel: per-sample noisy-weight 3x3 conv, data-parallel over 8 cores.

Z[b] = conv2d_valid(X[b], W * Werr[loc_id[b]]) + bias * Berr[loc_id[b]]

Shapes: X[32,64,64,64] f32, W[3,3,64,128], bias[128], Werr[1000,3,3,64,128],
Berr[1000,128], loc_id[32] i32 -> Z[32,62,62,128] f32.

Strategy: shard batch (4 samples/core). Host prep = layout only (X transpose
to cin-major + fp16 cast + two shifted stacks, gather of the 32 needed
Werr/Berr pool rows, weight packs). All FLOPs (memW = W*Werr, conv, bias)
run on device.

v5 design notes (140us baseline -> 82us -> 72.5us -> this):
  - X ships as TWO 128-partition fp16 "shifted stacks" per sample (xts1 =
    [X^T; X^T<<1], xts64 = [X^T; X^T<<64]), each split into an A tile
    (grid cols 0-2303, feeds chunks 0-3) and a B tile (cols 2048-4103,
    feeds chunks 4-7) on separate HWDGE rings. Separate tiles make the
    first chunk's data dependency a quarter of the X bytes, so the PE
    starts ~6us earlier than with whole-stack tiles.
  - 5 matmuls per 512-pixel PSUM chunk, all fp16 (1 cyc/row, ~2^-12 err):
    tap pairs (0,1),(64,65),(128,129) from xts1 at col offsets 0/64/128,
    pair (2,66) from xts64 at offset 2, single 130 as K=64 from the
    unshifted xts1 top half. Back-to-back matmuls pipeline at stream rate.
  - No on-chip transpose: z stays [cout, spatial] fp16; host does the
    final [spatial, cout] transpose + junk-column drop + f32 upcast.
  - All PSUM->SBUF copies (fused bias add) run on DVE via tensor_scalar;
    ScalarE issues no compute at all, which drops the activation-table
    loads from the preamble and leaves the ACT ring free for DMA.
  - z ships as four quarter-DMAs per sample on alternating rings, each
    issued the moment its two chunks are copied, so the output tail after
    the last matmul is ~1us.
"""

import sys
import numpy as np

for _p in ("/opt/trn_rl_repo", "/root/.axon_site"):
    if _p not in sys.path:
        sys.path.insert(0, _p)

N_CORES = 8
B = 32
PER_CORE = B // N_CORES
H = Wd = 64
CIN = 64
COUT = 128
HO = WO = 62
GRID = HO * WO          # 3844 valid output pixels (junk cols never stored)
XTL = 4104              # X^T grid cols: 4096 valid + pad (max read 4098)
ACOLS = 2304            # A tile: input cols [0, 2304), serves chunks 0-3
BOFF = 2048             # B tile: input cols [2048, 4104), serves chunks 4-7
BCOLS = XTL - BOFF      # 2056
CROWS = 8               # output grid rows per PSUM chunk
NCHUNK = CROWS * WO     # 496 valid pixels per chunk (junk-skipping rhs APs)
NCHUNKS = 8             # 7 full chunks + 1 of 6 rows (372 px)
WCOLS = 640             # 3 K=128 pair blocks + pair(2,66) block + K=64 blk 130

_compiled = {}


def _build():
    import concourse.bass as bass
    import concourse.mybir as mybir
    import concourse.tile as tile
    from concourse import bacc
    from concourse.bass import AP

    f32 = mybir.dt.float32
    f16 = mybir.dt.float16

    nc = bacc.Bacc("TRN2", target_bir_lowering=False, debug=False)

    x1a_in = nc.dram_tensor("x1a", [PER_CORE, 128, ACOLS], f16, kind="ExternalInput")
    x1b_in = nc.dram_tensor("x1b", [PER_CORE, 128, BCOLS], f16, kind="ExternalInput")
    x64a_in = nc.dram_tensor("x64a", [PER_CORE, 128, ACOLS], f16, kind="ExternalInput")
    x64b_in = nc.dram_tensor("x64b", [PER_CORE, 128, BCOLS], f16, kind="ExternalInput")
    w_in = nc.dram_tensor("w", [128, WCOLS], f16, kind="ExternalInput")
    g_in = nc.dram_tensor("g", [PER_CORE, 128, WCOLS], f16, kind="ExternalInput")
    bias_in = nc.dram_tensor("bias", [COUT, 1], f32, kind="ExternalInput")
    berr_in = nc.dram_tensor("berr", [COUT, PER_CORE], f32, kind="ExternalInput")
    z_out = nc.dram_tensor("z", [PER_CORE, 128, GRID], f16, kind="ExternalOutput")

    with tile.TileContext(nc) as tc:
        with (
            tc.tile_pool(name="const", bufs=1) as const,
            tc.tile_pool(name="xpool", bufs=2) as xpool,
            tc.tile_pool(name="wpool", bufs=2) as wpool,
            tc.tile_pool(name="zpool", bufs=2) as zpool,
            tc.tile_pool(name="psmm", bufs=6, space="PSUM") as psmm,
        ):
            w_t = const.tile([128, WCOLS], f16, tag="w")
            bias_t = const.tile([COUT, 1], f32, tag="bias")
            berr_t = const.tile([COUT, PER_CORE], f32, tag="berr")
            mb_all = const.tile([COUT, PER_CORE], f32, tag="mb")

            def load_sample(b, first=False):
                """DMA the X stack tiles + noise pack, form memW = W*G on DVE.

                The A tiles (first four chunks' data) go first on both
                rings; sample 0 additionally front-loads the small w/g
                packs so memW is ready before x1a lands.
                """
                x1at = xpool.tile([128, ACOLS], f16, tag="x1a")
                x1bt = xpool.tile([128, BCOLS], f16, tag="x1b")
                x64at = xpool.tile([128, ACOLS], f16, tag="x64a")
                x64bt = xpool.tile([128, BCOLS], f16, tag="x64b")
                gt = wpool.tile([128, WCOLS], f16, tag="gt")
                mw = wpool.tile([128, WCOLS], f16, tag="mw")
                if first:
                    # scalar ring starts the x64a bulk immediately; the tiny
                    # w/g packs ride ahead of x1a on the sync ring so memW is
                    # ready before the first stack lands
                    nc.scalar.dma_start(x64at[:], x64a_in[b])
                    nc.sync.dma_start(w_t[:], w_in[:])
                    nc.sync.dma_start(gt[:], g_in[b])
                    nc.sync.dma_start(x1at[:], x1a_in[b])
                    nc.vector.tensor_mul(mw[:], w_t[:], gt[:])
                    nc.scalar.dma_start(x64bt[:], x64b_in[b])
                    nc.sync.dma_start(x1bt[:], x1b_in[b])
                    nc.scalar.dma_start(bias_t[:], bias_in[:])
                    nc.scalar.dma_start(berr_t[:], berr_in[:])
                    nc.vector.tensor_scalar_mul(mb_all[:], berr_t[:], bias_t[:])
                else:
                    nc.sync.dma_start(x1at[:], x1a_in[b])
                    nc.scalar.dma_start(x64at[:], x64a_in[b])
                    nc.sync.dma_start(x1bt[:], x1b_in[b])
                    nc.scalar.dma_start(x64bt[:], x64b_in[b])
                    nc.scalar.dma_start(gt[:], g_in[b])
                    nc.vector.tensor_mul(mw[:], w_t[:], gt[:])
                return x1at, x1bt, x64at, x64bt, mw

            cur = load_sample(0, first=True)
            for b in range(PER_CORE):
                x1at, x1bt, x64at, x64bt, mw = cur
                if b + 1 < PER_CORE:
                    cur = load_sample(b + 1)

                zbuf = zpool.tile([128, GRID], f16, tag="zbuf")

                def rhs(xt, col, part, rows):
                    # junk-skipping moving AP: [part, rows, 62] reading the
                    # 64-wide input grid at +col, stepping 64 per output row
                    s = xt[0:part, col : col + 1]
                    return AP(s.tensor, s.offset, [list(s.ap[0]), [64, rows], [1, WO]])

                for c in range(NCHUNKS):
                    rows = min(CROWS, HO - c * CROWS)
                    n = rows * WO
                    base = c * NCHUNK
                    x1, x64 = (x1at, x64at) if c < 4 else (x1bt, x64bt)
                    off = c * CROWS * 64 - (0 if c < 4 else BOFF)
                    pc = psmm.tile([128, NCHUNK], f32, tag="pc")
                    # tap pairs (0,1), (64,65), (128,129): K=128 from xts1
                    for i in range(3):
                        nc.tensor.matmul(
                            pc[:, :n],
                            mw[:, i * 128 : (i + 1) * 128],
                            rhs(x1, off + i * 64, 128, rows),
                            start=(i == 0),
                            stop=False,
                        )
                    # tap pair (2,66): K=128 from xts64 at offset 2
                    nc.tensor.matmul(
                        pc[:, :n],
                        mw[:, 384:512],
                        rhs(x64, off + 2, 128, rows),
                        start=False,
                        stop=False,
                    )
                    # tap 130: K=64 from the unshifted xts1 top half
                    nc.tensor.matmul(
                        pc[:, :n],
                        mw[0:64, 512:640],
                        rhs(x1, off + 130, 64, rows),
                        start=False,
                        stop=True,
                    )
                    # PSUM -> SBUF with fused per-sample bias add on DVE
                    nc.vector.tensor_scalar_add(
                        zbuf[:, base : base + n], pc[:, :n], mb_all[:, b : b + 1]
                    )
                    # ship each finished output quarter immediately
                    if c % 2 == 1:
                        q = c // 2
                        lo, hi = q * 2 * NCHUNK, min((q + 1) * 2 * NCHUNK, GRID)
                        eng = nc.sync if q % 2 == 0 else nc.scalar
                        eng.dma_start(z_out[b][:, lo:hi], zbuf[:, lo:hi])

    nc.compile()
    return nc


def _get_nc():
    if "nc" not in _compiled:
        _compiled["nc"] = _build()
    return _compiled["nc"]


def _prep_inputs(X, W, bias, Werr, Berr, loc_id):
    """Host-side shard/layout prep. Returns per-core in_maps."""
    X = np.asarray(X, dtype=np.float32)
    W = np.asarray(W, dtype=np.float32)
    bias = np.asarray(bias, dtype=np.float32)
    Werr = np.asarray(Werr, dtype=np.float32)
    Berr = np.asarray(Berr, dtype=np.float32)
    loc_id = np.asarray(loc_id)

    # X^T stacks: x1 = [X^T; X^T shifted 1 col], x64 = [X^T; X^T shifted 64]
    xt = X.transpose(0, 3, 1, 2).reshape(B, CIN, H * Wd).astype(np.float16)
    x1 = np.zeros((B, 128, XTL), dtype=np.float16)
    x1[:, 0:64, : H * Wd] = xt
    x1[:, 64:128, : H * Wd - 1] = xt[:, :, 1:]
    x64 = np.zeros((B, 128, XTL), dtype=np.float16)
    x64[:, 0:64, : H * Wd] = xt
    x64[:, 64:128, : H * Wd - 64] = xt[:, :, 64:]

    def pack(w):
        # w: [..., 3, 3, 64, 128] -> [..., 128, 640]
        lead = w.shape[:-4]
        p = np.zeros(lead + (128, WCOLS), dtype=np.float16)
        for fh in range(3):  # K=128 pair blocks: taps (fh,0) + (fh,1)
            p[..., 0:64, fh * 128 : (fh + 1) * 128] = w[..., fh, 0, :, :]
            p[..., 64:128, fh * 128 : (fh + 1) * 128] = w[..., fh, 1, :, :]
        p[..., 0:64, 384:512] = w[..., 0, 2, :, :]   # tap 2 (xts64 top)
        p[..., 64:128, 384:512] = w[..., 1, 2, :, :]  # tap 66 (xts64 bottom)
        p[..., 0:64, 512:640] = w[..., 2, 2, :, :]   # tap 130 (K=64)
        return p

    wpack = pack(W)
    gpack = pack(Werr[loc_id])  # [B, 128, 640]

    be = Berr[loc_id]  # [B, 128]
    bias_col = np.ascontiguousarray(bias.reshape(COUT, 1))

    in_maps = []
    for i in range(N_CORES):
        s = slice(i * PER_CORE, (i + 1) * PER_CORE)
        in_maps.append(
            {
                "x1a": np.ascontiguousarray(x1[s, :, :ACOLS]),
                "x1b": np.ascontiguousarray(x1[s, :, BOFF:]),
                "x64a": np.ascontiguousarray(x64[s, :, :ACOLS]),
                "x64b": np.ascontiguousarray(x64[s, :, BOFF:]),
                "w": wpack,
                "g": np.ascontiguousarray(gpack[s]),
                "bias": bias_col,
                "berr": np.ascontiguousarray(be[s].T),
            }
        )
    return in_maps


def _run(in_maps, trace=False, **kw):
    from concourse.bass_utils import run_bass_kernel_spmd

    nc = _get_nc()
    return run_bass_kernel_spmd(nc, in_maps, list(range(N_CORES)), trace=trace, **kw)


def _unshard(results):
    zb = np.concatenate([results[i]["z"] for i in range(N_CORES)], axis=0)
    # zb[b, cout, 3844] fp16 -> Z[b, ho, wo, cout] f32
    v = zb.astype(np.float32).reshape(B, COUT, HO, WO).transpose(0, 2, 3, 1)
    return np.ascontiguousarray(v)


def kernel(X, W, bias, Werr, Berr, loc_id):
    in_maps = _prep_inputs(X, W, bias, Werr, Berr, loc_id)
    res = _run(in_maps)
    return _unshard(res.results)


# revision 22
# speedup vs baseline: 2.4872x; 1.1411x over previous
"""ConvAConnect TRN2 kernel: per-sample noisy-weight 3x3 conv, data-parallel over 8 cores.

Z[b] = conv2d_valid(X[b], W * Werr[loc_id[b]]) + bias * Berr[loc_id[b]]

Shapes: X[32,64,64,64] f32, W[3,3,64,128], bias[128], Werr[1000,3,3,64,128],
Berr[1000,128], loc_id[32] i32 -> Z[32,62,62,128] f32.

Strategy: shard batch (4 samples/core). Host prep = layout only (X transpose
to cin-major + fp16 cast + two shifted stacks, gather of the 32 needed
Werr/Berr pool rows, weight packs). All FLOPs (memW = W*Werr, conv, bias)
run on device.

v5 design notes (140us baseline -> 82us -> 72.5us -> this):
  - X ships as TWO 128-partition fp16 "shifted stacks" per sample (xts1 =
    [X^T; X^T<<1], xts64 = [X^T; X^T<<64]), each split into an A tile
    (grid cols 0-2303, feeds chunks 0-3) and a B tile (cols 2048-4103,
    feeds chunks 4-7) on separate HWDGE rings. Separate tiles make the
    first chunk's data dependency a quarter of the X bytes, so the PE
    starts ~6us earlier than with whole-stack tiles.
  - 5 matmuls per 512-pixel PSUM chunk, all fp16 (1 cyc/row, ~2^-12 err):
    tap pairs (0,1),(64,65),(128,129) from xts1 at col offsets 0/64/128,
    pair (2,66) from xts64 at offset 2, single 130 as K=64 from the
    unshifted xts1 top half. Back-to-back matmuls pipeline at stream rate.
  - No on-chip transpose: z stays [cout, spatial] fp16; host does the
    final [spatial, cout] transpose + junk-column drop + f32 upcast.
  - All PSUM->SBUF copies (fused bias add) run on DVE via tensor_scalar;
    ScalarE issues no compute at all, which drops the activation-table
    loads from the preamble and leaves the ACT ring free for DMA.
  - z ships as four quarter-DMAs per sample on alternating rings, each
    issued the moment its two chunks are copied, so the output tail after
    the last matmul is ~1us.
"""

import sys
import numpy as np

for _p in ("/opt/trn_rl_repo", "/root/.axon_site"):
    if _p not in sys.path:
        sys.path.insert(0, _p)

N_CORES = 8
B = 32
PER_CORE = B // N_CORES
H = Wd = 64
CIN = 64
COUT = 128
HO = WO = 62
GRID = HO * WO          # 3844 valid output pixels (junk cols never stored)
XTL = 4104              # X^T grid cols: 4096 valid + pad (max read 4098)
# X stacks ship in 3 overlapping column pieces so the first chunk's data
# dependency is only 1152 cols; piece p serves chunks CHUNK_PIECE[c]
POFF = (0, 1024, 2048)  # piece start col
PCOLS = (1152, 1152, XTL - 2048)
CHUNK_PIECE = (0, 0, 1, 1, 2, 2, 2, 2)
CROWS = 8               # output grid rows per PSUM chunk
NCHUNK = CROWS * WO     # 496 valid pixels per chunk (junk-skipping rhs APs)
NCHUNKS = 8             # 7 full chunks + 1 of 6 rows (372 px)
WCOLS = 640             # 3 K=128 pair blocks + pair(2,66) block + K=64 blk 130

_compiled = {}


def _build():
    import concourse.bass as bass
    import concourse.mybir as mybir
    import concourse.tile as tile
    from concourse import bacc
    from concourse.bass import AP

    f32 = mybir.dt.float32
    f16 = mybir.dt.float16

    nc = bacc.Bacc("TRN2", target_bir_lowering=False, debug=False)

    x1_in = [
        nc.dram_tensor(f"x1p{p}", [PER_CORE, 128, PCOLS[p]], f16, kind="ExternalInput")
        for p in range(3)
    ]
    x64_in = [
        nc.dram_tensor(f"x64p{p}", [PER_CORE, 128, PCOLS[p]], f16, kind="ExternalInput")
        for p in range(3)
    ]
    w_in = nc.dram_tensor("w", [128, WCOLS], f16, kind="ExternalInput")
    g_in = nc.dram_tensor("g", [PER_CORE, 128, WCOLS], f16, kind="ExternalInput")
    bias_in = nc.dram_tensor("bias", [COUT, 1], f32, kind="ExternalInput")
    berr_in = nc.dram_tensor("berr", [COUT, PER_CORE], f32, kind="ExternalInput")
    z_out = nc.dram_tensor("z", [PER_CORE, 128, GRID], f16, kind="ExternalOutput")

    with tile.TileContext(nc) as tc:
        with (
            tc.tile_pool(name="const", bufs=1) as const,
            tc.tile_pool(name="xpool", bufs=2) as xpool,
            tc.tile_pool(name="wpool", bufs=2) as wpool,
            tc.tile_pool(name="zpool", bufs=2) as zpool,
            tc.tile_pool(name="psmm", bufs=6, space="PSUM") as psmm,
        ):
            w_t = const.tile([128, WCOLS], f16, tag="w")
            bias_t = const.tile([COUT, 1], f32, tag="bias")
            berr_t = const.tile([COUT, PER_CORE], f32, tag="berr")
            mb_all = const.tile([COUT, PER_CORE], f32, tag="mb")

            def load_sample(b, first=False):
                """DMA the X stack pieces + noise pack, form memW = W*G on DVE.

                Piece 0 (first two chunks' data) leads on both rings; for
                sample 0 the tiny g/w packs ride ahead so memW is ready by
                the time piece 0 lands.
                """
                x1t = [
                    xpool.tile([128, PCOLS[p]], f16, tag=f"x1p{p}", name=f"x1p{p}_t")
                    for p in range(3)
                ]
                x64t = [
                    xpool.tile([128, PCOLS[p]], f16, tag=f"x64p{p}", name=f"x64p{p}_t")
                    for p in range(3)
                ]
                gt = wpool.tile([128, WCOLS], f16, tag="gt")
                mw = wpool.tile([128, WCOLS], f16, tag="mw")
                if first:
                    nc.sync.dma_start(gt[:], g_in[b])
                    nc.scalar.dma_start(w_t[:], w_in[:])
                    for p in range(2):
                        nc.sync.dma_start(x1t[p][:], x1_in[p][b])
                        nc.scalar.dma_start(x64t[p][:], x64_in[p][b])
                    nc.vector.tensor_mul(mw[:], w_t[:], gt[:])
                    nc.sync.dma_start(bias_t[:], bias_in[:])
                    nc.sync.dma_start(berr_t[:], berr_in[:])
                    nc.vector.tensor_scalar_mul(mb_all[:], berr_t[:], bias_t[:])
                    nc.sync.dma_start(x1t[2][:], x1_in[2][b])
                    nc.scalar.dma_start(x64t[2][:], x64_in[2][b])
                else:
                    for p in range(3):
                        nc.sync.dma_start(x1t[p][:], x1_in[p][b])
                        nc.scalar.dma_start(x64t[p][:], x64_in[p][b])
                    nc.scalar.dma_start(gt[:], g_in[b])
                    nc.vector.tensor_mul(mw[:], w_t[:], gt[:])
                return x1t, x64t, mw

            cur = load_sample(0, first=True)
            for b in range(PER_CORE):
                x1t, x64t, mw = cur
                if b + 1 < PER_CORE:
                    cur = load_sample(b + 1)

                zbuf = zpool.tile([128, GRID], f16, tag="zbuf")

                def rhs(xt, col, part, rows):
                    # junk-skipping moving AP: [part, rows, 62] reading the
                    # 64-wide input grid at +col, stepping 64 per output row
                    s = xt[0:part, col : col + 1]
                    return AP(s.tensor, s.offset, [list(s.ap[0]), [64, rows], [1, WO]])

                for c in range(NCHUNKS):
                    rows = min(CROWS, HO - c * CROWS)
                    n = rows * WO
                    base = c * NCHUNK
                    piece = CHUNK_PIECE[c]
                    x1, x64 = x1t[piece], x64t[piece]
                    off = c * CROWS * 64 - POFF[piece]
                    pc = psmm.tile([128, NCHUNK], f32, tag="pc")
                    # tap pairs (0,1), (64,65), (128,129): K=128 from xts1
                    for i in range(3):
                        nc.tensor.matmul(
                            pc[:, :n],
                            mw[:, i * 128 : (i + 1) * 128],
                            rhs(x1, off + i * 64, 128, rows),
                            start=(i == 0),
                            stop=False,
                        )
                    # tap pair (2,66): K=128 from xts64 at offset 2
                    nc.tensor.matmul(
                        pc[:, :n],
                        mw[:, 384:512],
                        rhs(x64, off + 2, 128, rows),
                        start=False,
                        stop=False,
                    )
                    # tap 130: uniform K=128 (bottom-half weights are zero,
                    # so the shifted rows contribute nothing)
                    nc.tensor.matmul(
                        pc[:, :n],
                        mw[:, 512:640],
                        rhs(x1, off + 130, 128, rows),
                        start=False,
                        stop=True,
                    )
                    # PSUM -> SBUF with fused per-sample bias add on DVE
                    nc.vector.tensor_scalar_add(
                        zbuf[:, base : base + n], pc[:, :n], mb_all[:, b : b + 1]
                    )
                    # ship each finished output quarter immediately
                    if c % 2 == 1:
                        q = c // 2
                        lo, hi = q * 2 * NCHUNK, min((q + 1) * 2 * NCHUNK, GRID)
                        eng = nc.sync if q % 2 == 0 else nc.scalar
                        eng.dma_start(z_out[b][:, lo:hi], zbuf[:, lo:hi])

    nc.compile()
    return nc


def _get_nc():
    if "nc" not in _compiled:
        _compiled["nc"] = _build()
    return _compiled["nc"]


def _prep_inputs(X, W, bias, Werr, Berr, loc_id):
    """Host-side shard/layout prep. Returns per-core in_maps."""
    X = np.asarray(X, dtype=np.float32)
    W = np.asarray(W, dtype=np.float32)
    bias = np.asarray(bias, dtype=np.float32)
    Werr = np.asarray(Werr, dtype=np.float32)
    Berr = np.asarray(Berr, dtype=np.float32)
    loc_id = np.asarray(loc_id)

    # X^T stacks: x1 = [X^T; X^T shifted 1 col], x64 = [X^T; X^T shifted 64]
    xt = X.transpose(0, 3, 1, 2).reshape(B, CIN, H * Wd).astype(np.float16)
    x1 = np.zeros((B, 128, XTL), dtype=np.float16)
    x1[:, 0:64, : H * Wd] = xt
    x1[:, 64:128, : H * Wd - 1] = xt[:, :, 1:]
    x64 = np.zeros((B, 128, XTL), dtype=np.float16)
    x64[:, 0:64, : H * Wd] = xt
    x64[:, 64:128, : H * Wd - 64] = xt[:, :, 64:]

    def pack(w):
        # w: [..., 3, 3, 64, 128] -> [..., 128, 640]
        lead = w.shape[:-4]
        p = np.zeros(lead + (128, WCOLS), dtype=np.float16)
        for fh in range(3):  # K=128 pair blocks: taps (fh,0) + (fh,1)
            p[..., 0:64, fh * 128 : (fh + 1) * 128] = w[..., fh, 0, :, :]
            p[..., 64:128, fh * 128 : (fh + 1) * 128] = w[..., fh, 1, :, :]
        p[..., 0:64, 384:512] = w[..., 0, 2, :, :]   # tap 2 (xts64 top)
        p[..., 64:128, 384:512] = w[..., 1, 2, :, :]  # tap 66 (xts64 bottom)
        p[..., 0:64, 512:640] = w[..., 2, 2, :, :]   # tap 130 (K=64)
        return p

    wpack = pack(W)
    gpack = pack(Werr[loc_id])  # [B, 128, 640]

    be = Berr[loc_id]  # [B, 128]
    bias_col = np.ascontiguousarray(bias.reshape(COUT, 1))

    in_maps = []
    for i in range(N_CORES):
        s = slice(i * PER_CORE, (i + 1) * PER_CORE)
        m = {
            "w": wpack,
            "g": np.ascontiguousarray(gpack[s]),
            "bias": bias_col,
            "berr": np.ascontiguousarray(be[s].T),
        }
        for p in range(3):
            lo, hi = POFF[p], POFF[p] + PCOLS[p]
            m[f"x1p{p}"] = np.ascontiguousarray(x1[s, :, lo:hi])
            m[f"x64p{p}"] = np.ascontiguousarray(x64[s, :, lo:hi])
        in_maps.append(m)
    return in_maps


def _run(in_maps, trace=False, **kw):
    from concourse.bass_utils import run_bass_kernel_spmd

    nc = _get_nc()
    return run_bass_kernel_spmd(nc, in_maps, list(range(N_CORES)), trace=trace, **kw)


def _unshard(results):
    zb = np.concatenate([results[i]["z"] for i in range(N_CORES)], axis=0)
    # zb[b, cout, 3844] fp16 -> Z[b, ho, wo, cout] f32
    v = zb.astype(np.float32).reshape(B, COUT, HO, WO).transpose(0, 2, 3, 1)
    return np.ascontiguousarray(v)


def kernel(X, W, bias, Werr, Berr, loc_id):
    in_maps = _prep_inputs(X, W, bias, Werr, Berr, loc_id)
    res = _run(in_maps)
    return _unshard(res.results)
